# revision 19
# baseline (speedup 1.0000x reference)
"""Memory-efficient Dice loss on 8 Trainium2 NeuronCores.

Full inputs:
  logits  (2, 16, 64, 128, 128) fp32
  targets (2, 64, 128, 128) int  (values 0..15)
Output: scalar fp32 loss = 1 - mean_{b, c != 0} dice[b, c].

Sharding: 8 cores over (B=2) x (D quartered into 4 slabs of 16).

Interleaved class-quota layout: the host sorts each core's voxels by
target class into columns of 128 with a FIXED quota of 128 columns
per class, interleaved so global column j < 2048 always holds class
j % 16; 64 overflow columns at the end absorb classes whose count
exceeds the quota (per-column single class, host keeps the map).
Deficit columns are padded with dummy voxels (all-class logits = 0)
whose exact 1/16 contributions are subtracted on the host. Per-column
storage is class-interleaved too ([p][col][c]), which makes BOTH
reductions affine:

  e = exp(logits) bf16 (one contiguous full-rate ACT pass per block);
  Z = sum_c e: 4 halving tensor_tensor levels over the contiguous
    16-class groups (packed bf16); r = 1/Z (fp32 custom op) -> bf16.
  PS: per 32-col chunk, stationary = r-chunk [P,32], moving =
    e [P,16,32] (strides 1,16); PSUM diag out[g, 16g'+c ...] ->
    out[g, c*32+g] accumulates probs_sum partials over all 66 chunks.
  I:  per 128-col quota chunk, stationary = r [P,128], moving = the
    target-class plane e_t [P,8,16] via a stepped slice (0:256:17 of
    the 256-elem column-pair groups); ONE [128,128] PSUM diag
    accumulates all 16 chunks; diag position g has class g % 16.
    Only the 64 overflow columns keep a tiny lt path (exp(lt_ov) * r
    summed by a ones-stationary matmul).

Logits ship as fp8 e4m3 (TRN-native ml_dtypes.float8_e4m3: measured
end-to-end rel err ~5e-6 << 2e-2), quartering HBM bytes; each block
is ONE dma_start ([P, C*BW] contiguous both sides) since a single
descriptor is split across all 16 SDMA engines. Scalar (ACT) is the
bottleneck engine (exp floor = C*M cols / 1.2GHz ~= 28us): it runs
exactly one contiguous ACTIVATE per block. 7 uneven blocks (128-col
first, 192-col last) shorten the DMA ramp and the tree->recip->
matmul tail; the last block's DVE/PE chain is j-split.
"""

import ml_dtypes
import numpy as np

import concourse.bass as bass
import concourse.mybir as mybir
import concourse.tile as tile
from concourse import bacc
from concourse.bass_utils import run_bass_kernel_spmd

B, C, D, H, W = 2, 16, 64, 128, 128
P = 128            # SBUF partitions
NCORES = 8
DSH = D // 4       # d-planes per core
N = DSH * H * W    # real voxels per core = 262144
Q = 128            # quota columns per class (col j<2048 holds class j%16)
OV = 64            # overflow columns (host-mapped classes)
M = C * Q + OV     # 2112 columns per core
G = 32             # columns per PS-matmul chunk (PSUM: 16*G <= 512)
IW = 128           # columns per I-matmul chunk
NCH = M // G       # 66 PS chunks
NICH = C * Q // IW # 16 I chunks
# block sizes: multiples of G; I-matmul pieces adapt to block bounds.
# The 64 overflow columns lead (block 0) so their lt path retires early;
# quota columns span [OV, M) and global column V holds class (V-OV) % 16.
BLOCKS = [64, 256, 384, 384, 384, 384, 160, 96]
assert sum(BLOCKS) == M and all(b % G == 0 for b in BLOCKS)
BOFF = [int(x) for x in np.concatenate([[0], np.cumsum(BLOCKS)])]
NBLK = len(BLOCKS)
BWMAX = max(BLOCKS)

SMOOTH = 1.0
IGNORE_INDEX = 0


def build():
    fp32 = mybir.dt.float32
    bf16 = mybir.dt.bfloat16
    fp8 = mybir.dt.float8e4
    AL = mybir.AluOpType
    X = mybir.ActivationFunctionType.Exp

    nc = bacc.Bacc("TRN2", target_bir_lowering=False, debug=False)
    logits_d = [
        nc.dram_tensor(f"l{b}", [P, C * BLOCKS[b]], fp8, kind="ExternalInput")
        for b in range(NBLK)
    ]
    ltov_d = nc.dram_tensor("ltov", [P, OV], bf16, kind="ExternalInput")
    # packed output: [0:32, 0:512]=PS diag mat, [:, 512:640]=I diag mat,
    # [0:1, 640:704]=overflow column sums
    out_d = nc.dram_tensor("o", [P, 704], fp32, kind="ExternalOutput")

    with (
        tile.TileContext(nc) as tc,
        tc.tile_pool(name="main", bufs=1) as pool,
        tc.tile_pool(name="psum", bufs=1, space="PSUM") as psump,
    ):
        def tcT(shape, dtype, name, pl=None):
            return (pl or pool).tile(shape, dtype, name=name, tag=name)

        NBUF = 3
        NLBUF = 4
        Lb = [tcT([P, C * BWMAX], fp8, name=f"Lb{i}") for i in range(NLBUF)]
        Ew = [tcT([P, C * BWMAX], bf16, name=f"Ew{i}") for i in range(NBUF)]
        rbl = [tcT([P, BWMAX], bf16, name=f"rb{i}") for i in range(NBUF)]
        zw = tcT([P, 8 * BWMAX], bf16, name="zw")
        rf = tcT([P, BWMAX], fp32, name="rf")
        zf = tcT([P, BWMAX], fp32, name="zf")
        onesw = tcT([P, 1], bf16, name="onesw")
        ltov = tcT([P, OV], bf16, name="ltov")
        etov = tcT([P, OV], bf16, name="etov")
        gov = tcT([P, OV], bf16, name="gov")
        osb = tcT([P, 704], fp32, name="osb")
        acc = tcT([G, C * G], fp32, name="acc", pl=psump)
        acc2 = tcT([P, IW], fp32, name="acc2", pl=psump)
        acch = tcT([1, OV], fp32, name="acch", pl=psump)

        # preamble: exp table preload + block-0/ltov DMAs first
        nc.scalar.activation(zw[:, 0:1], zw[:, 0:1], X)
        nc.sync.dma_start(Lb[0][:, 0 : C * BLOCKS[0]], logits_d[0].ap())
        nc.sync.dma_start(ltov[:], ltov_d.ap())
        nc.vector.memset(onesw[:], 1.0)
        nc.vector.memset(osb[:], 0.0)

        tt = nc.vector.tensor_tensor
        for b in range(NBLK):
            i = b % NBUF
            bw = BLOCKS[b]
            if b > 0:
                nc.sync.dma_start(Lb[b % NLBUF][:, 0 : C * bw], logits_d[b].ap())

            # one contiguous full-rate exp per block
            nc.scalar.activation(
                Ew[i][:, 0 : C * bw], Lb[b % NLBUF][:, 0 : C * bw], X
            )
            if b == 0:
                nc.scalar.activation(etov[:], ltov[:], X)

            # hybrid layout: [p][a (16-col group)][c (class)][j1 (col%16)]
            flat = Ew[i][:, 0 : C * bw]
            E4 = flat.rearrange("p (a c j) -> p a c j", c=C, j=16)
            Epr = flat.rearrange("p (a r) -> p a r", r=16 * C)  # [P,bw/16,256]
            z4 = zw[:, 0 : 8 * bw].rearrange("p (a s j) -> p a s j", s=8, j=16)

            for (j0, j1) in ((0, bw),):
                ah = slice(j0 // 16, j1 // 16)
                jh = slice(j0, j1)
                # Z = sum_c e: 4 halving levels, 16-elem contiguous runs
                tt(z4[:, ah, :, :], E4[:, ah, 0:8, :], E4[:, ah, 8:16, :], AL.add)
                tt(z4[:, ah, 0:4, :], z4[:, ah, 0:4, :], z4[:, ah, 4:8, :], AL.add)
                tt(z4[:, ah, 0:2, :], z4[:, ah, 0:2, :], z4[:, ah, 2:4, :], AL.add)
                tt(zf[:, jh], z4[:, ah, 0, :], z4[:, ah, 1, :], AL.add)
                nc.vector.reciprocal_approx_fast(rf[:, jh], zf[:, jh])
                nc.vector.tensor_copy(rbl[i][:, jh], rf[:, jh])
                # PS matmuls: stationary = r-chunk bf16, moving = the
                # chunk's contiguous [P, G*C] slab; partials land at
                # out[g, 256*(g//16) + 16*c + g%16]
                for k in range(j0 // G, j1 // G):
                    m = BOFF[b] // G + k
                    jc = k * G
                    nc.tensor.matmul(
                        acc[:, :],
                        rbl[i][:, jc : jc + G],
                        flat[:, jc * C : (jc + G) * C],
                        start=m == 0,
                        stop=m == NCH - 1,
                        skip_group_check=True,
                    )
            if b == 0:
                # overflow intersection: g = exp(lt_ov)*r, column sums by PE
                tt(gov[:], etov[:], rbl[0][:, 0:OV], AL.mult)
                nc.tensor.matmul(
                    acch[:, :], onesw[:, 0:1], gov[:, :],
                    start=True, stop=True, skip_group_check=True,
                )
                nc.vector.tensor_copy(osb[0:1, C * G + IW :], acch[0:1, :])
                continue
            # I matmuls: stationary = r piece [P,w<=128], moving = the
            # target-class plane (stepped slice); diag cell g accumulates
            # class g%16 regardless of the piece's global offset
            lo = 0
            while lo < bw:
                w = min(IW, bw - lo)
                nc.tensor.matmul(
                    acc2[0:w, 0:w],
                    rbl[i][:, lo : lo + w],
                    Epr[:, lo // 16 : (lo + w) // 16, 0 : 16 * C : C + 1],
                    start=BOFF[b] + lo == OV,
                    stop=BOFF[b] + lo + w == M,
                    skip_group_check=True,
                )
                lo += w

        # drain PSUM -> SBUF on the (now idle) Scalar engine -> one DMA
        nc.scalar.copy(osb[0:G, 0 : C * G], acc[:])
        nc.scalar.copy(osb[:, C * G : C * G + IW], acc2[:])
        nc.sync.dma_start(out_d.ap(), osb[:])
    nc.compile()
    return nc


_NC_CACHE = {}


def _get_nc():
    if "nc" not in _NC_CACHE:
        _NC_CACHE["nc"] = build()
    return _NC_CACHE["nc"]


def _prep_core(lg, t):
    """lg [C, N] fp32, t [N] int -> device inputs + host metadata."""
    cnts = np.bincount(t, minlength=C)
    order = np.argsort(t, kind="stable")
    offs = np.concatenate([[0], np.cumsum(cnts)])

    vox = np.full(M * P, -1, dtype=np.int64)   # (column j, slot p) major
    qdum = np.zeros(C, dtype=np.int64)         # dummies inside class quotas
    ov_cm = np.zeros(OV, dtype=np.int64)       # overflow column class map
    ovdum = np.zeros(C, dtype=np.int64)        # dummies in overflow columns
    ov_col = 0
    for c in range(C):
        n_c = int(cnts[c])
        n_q = min(n_c, Q * P)
        idx = np.arange(n_q)
        # class c quota voxel a*P+p -> global column OV + 16*a + c, slot p
        vox[(OV + C * (idx // P) + c) * P + idx % P] = order[
            offs[c] : offs[c] + n_q
        ]
        qdum[c] = Q * P - n_q
        n_x = n_c - n_q
        if n_x > 0:
            ncols = (n_x + P - 1) // P
            base = ov_col * P
            vox[base : base + n_x] = order[offs[c] + n_q : offs[c] + n_c]
            ov_cm[ov_col : ov_col + ncols] = c
            ovdum[c] += ncols * P - n_x
            ov_col += ncols
    assert ov_col <= OV, f"overflow region too small: {ov_col} > {OV}"
    ovdum[0] += (OV - ov_col) * P  # trailing all-dummy columns, class 0

    mask = vox >= 0
    A = lg[:, np.clip(vox, 0, None)]  # [C, M*P]
    A[:, ~mask] = 0.0
    A3 = A.reshape(C, M, P)
    im = {}
    for b in range(NBLK):
        bw = BLOCKS[b]
        # hybrid layout [p][a][c][j1]: a = 16-col group, j1 = col % 16
        seg = A3[:, BOFF[b] : BOFF[b + 1], :].reshape(C, bw // 16, 16, P)
        im[f"l{b}"] = np.ascontiguousarray(
            seg.transpose(3, 1, 0, 2)
        ).reshape(P, C * bw).astype(ml_dtypes.float8_e4m3)
    ovj = np.arange(OV)
    ltov = A3[ov_cm, ovj, :]  # [OV, P] target-class logits of ov columns
    im["ltov"] = np.ascontiguousarray(ltov.T).astype(ml_dtypes.bfloat16)
    return im, (qdum, ov_cm, ovdum, cnts)


def shard_inputs(logits, targets):
    """Core i gets batch i//4, d-slab i%4. Returns (in_maps, metas)."""
    in_maps, metas = [], []
    for i in range(NCORES):
        b, q = divmod(i, 4)
        lg = np.ascontiguousarray(
            logits[b, :, q * DSH : (q + 1) * DSH], dtype=np.float32
        ).reshape(C, N)
        t = np.ascontiguousarray(
            targets[b, q * DSH : (q + 1) * DSH], dtype=np.int64
        ).reshape(N)
        im, meta = _prep_core(lg, t)
        in_maps.append(im)
        metas.append(meta)
    return in_maps, metas


def _core_stats(res, meta):
    """Per-core (I, PS, counts) from device outputs + host metadata."""
    qdum, ov_cm, ovdum, cnts = meta
    o = res["o"].astype(np.float64)
    gidx = np.arange(G)
    fbase = (gidx // 16) * 256 + gidx % 16  # chunk-slab offset of column g
    PS = o[gidx[:, None], fbase[:, None] + 16 * np.arange(C)[None, :]].sum(axis=0)
    PS -= (qdum.sum() + ovdum.sum()) / 16.0  # dummies add e*r = 1/16 everywhere
    d2 = o[np.arange(P), C * G + np.arange(IW)]  # I diag, class g%16
    I = d2.reshape(-1, C).sum(axis=0) - qdum / 16.0
    hov = o[0, C * G + IW :]
    I += np.bincount(ov_cm, weights=hov, minlength=C)[:C] - ovdum / 16.0
    return I, PS, cnts.astype(np.float64)


def kernel(logits, targets):
    logits = np.asarray(logits)
    targets = np.asarray(targets)
    nc = _get_nc()
    in_maps, metas = shard_inputs(logits, targets)
    res = run_bass_kernel_spmd(nc, in_maps, list(range(NCORES))).results
    inter = np.zeros((B, C))
    probs_sum = np.zeros((B, C))
    counts = np.zeros((B, C))
    for i in range(NCORES):
        I, PS, CNT = _core_stats(res[i], metas[i])
        inter[i // 4] += I
        probs_sum[i // 4] += PS
        counts[i // 4] += CNT
    dice = (2.0 * inter + SMOOTH) / (probs_sum + counts + SMOOTH)
    mask = np.ones(C)
    mask[IGNORE_INDEX] = 0.0
    mean_dice = (dice * mask[None, :]).sum() / (B * (C - 1))
    return np.float32(1.0 - mean_dice)


# revision 28
# speedup vs baseline: 1.1147x; 1.1147x over previous
"""Memory-efficient Dice loss on 8 Trainium2 NeuronCores.

Full inputs:
  logits  (2, 16, 64, 128, 128) fp32
  targets (2, 64, 128, 128) int  (values 0..15)
Output: scalar fp32 loss = 1 - mean_{b, c != 0} dice[b, c].

Sharding: 8 cores over (B=2) x (D quartered into 4 slabs of 16).

Hybrid class-quota layout: the host sorts each core's voxels by target
class into columns of 128. 64 overflow columns lead (block 0,
host-mapped classes via a tiny lt plane); the 2048 quota columns
follow with a FIXED schedule: global column V holds class (V-64)%16.
Deficit columns are padded with dummy voxels (all-class logits = 0)
whose exact 1/16 contributions are subtracted on the host. Per-block
on-chip storage is [p][a (16-col group)][c (class)][j1 (col%16)], so
every reduction is an affine access pattern:

  e = exp(logits) bf16: ONE contiguous full-rate ACTIVATE per block
    (Scalar is the bottleneck engine; exp floor = C*M/1.2GHz ~= 28us).
  Z = sum_c e: 4 halving tensor_tensor levels with 16-elem contiguous
    runs (packed bf16, DVE); r = 1/Z (fp32 custom op) -> bf16.
  PS (probs_sum): per 32-col chunk, stationary = r-chunk [P,32],
    moving = the chunk's contiguous [P, 512] slab; PSUM partials land
    at out[g, 256*(g//16) + 16*c + g%16], accumulated over 66 chunks.
  I (intersection): per <=128-col piece, stationary = r piece, moving
    = the target-class plane e_t via a stepped slice (0:256:17 of the
    256-elem groups); ONE [128,128] PSUM diag accumulates everything
    (diag cell g always has class g%16 since pieces start %32==0).
  Overflow: g = exp(lt_ov)*r, column sums via a ones-stationary
    matmul, host bins by its column->class map.

Logits ship as fp8 e4m3 (TRN-native ml_dtypes.float8_e4m3; softmax's
ratio structure damps the quantization - measured end-to-end rel err
~5e-6 << 2e-2), quartering HBM bytes; each block is ONE dma_start
(contiguous both sides - a single descriptor is split across all 16
SDMA engines). Raw hand-scheduled bass (no TileContext): ~13 manual
semaphores with embedded waits/incs replace the auto-scheduler's ~254
(whose serialized end-of-kernel resets alone cost ~8us), and each
engine's FIFO carries only real work. Cross-engine edges:
  sd[b] (DMA b landed, +16) -> exp(b); s_act (exps done) -> tree L1,
  Lb reuse, etov; s_dve (cast b done) -> block b matmuls (transitively
  orders exp/tree too); s_pe (block b tensor ops done) -> Ew/rbl reuse
  (standalone waits on Scalar/Vector for b>=3) and the final drains;
  s_gov -> overflow h-matmul; s_drain -> output DMA.
"""

import ml_dtypes
import numpy as np

import concourse.bass as bass
import concourse.mybir as mybir
from concourse import bacc
from concourse.bass_utils import run_bass_kernel_spmd

B, C, D, H, W = 2, 16, 64, 128, 128
P = 128            # SBUF partitions
NCORES = 8
DSH = D // 4       # d-planes per core
N = DSH * H * W    # real voxels per core = 262144
Q = 128            # quota columns per class
OV = 64            # overflow columns (host-mapped classes), block 0
M = C * Q + OV     # 2112 columns per core
G = 32             # columns per PS-matmul chunk (PSUM: 16*G <= 512)
IW = 128           # max columns per I-matmul piece
NCH = M // G       # 66 PS chunks
BLOCKS = [64, 320, 384, 384, 384, 384, 192]
assert sum(BLOCKS) == M and all(b % G == 0 for b in BLOCKS)
BOFF = [int(x) for x in np.concatenate([[0], np.cumsum(BLOCKS)])]
NBLK = len(BLOCKS)
BWMAX = max(BLOCKS)
NBUF = 3           # Ew / rbl buffers
NLBUF = 4          # Lb buffers

SMOOTH = 1.0
IGNORE_INDEX = 0


def build():
    from contextlib import ExitStack

    fp32 = mybir.dt.float32
    bf16 = mybir.dt.bfloat16
    fp8 = mybir.dt.float8e4
    AL = mybir.AluOpType
    X = mybir.ActivationFunctionType.Exp

    nc = bacc.Bacc("TRN2", target_bir_lowering=False, debug=False)
    logits_d = [
        nc.dram_tensor(f"l{b}", [P, C * BLOCKS[b]], fp8, kind="ExternalInput")
        for b in range(NBLK)
    ]
    ltov_d = nc.dram_tensor("ltov", [P, OV], bf16, kind="ExternalInput")
    # packed output: [0:32, 0:512]=PS matrix, [:, 512:640]=I diag matrix,
    # [0:1, 640:704]=overflow column sums
    out_d = nc.dram_tensor("o", [P, 704], fp32, kind="ExternalOutput")

    es = ExitStack()
    with es:
        sem = lambda name: es.enter_context(nc.semaphore(name))
        sbt = lambda name, shape, dt: es.enter_context(
            nc.sbuf_tensor(name, shape, dt)
        )
        pst = lambda name, shape, dt: es.enter_context(
            nc.psum_tensor(name, shape, dt)
        )
        sd = [sem(f"sd{b}") for b in range(NBLK)]
        sdL = sem("sdL")
        s_act = sem("s_act")
        s_dve = sem("s_dve")
        s_pe = sem("s_pe")
        s_gov = sem("s_gov")
        s_drain = sem("s_drain")
        s_nul = sem("s_nul")  # scrap target so pure waits carry an update

        Lb = [sbt(f"Lb{i}", [P, C * BWMAX], fp8) for i in range(NLBUF)]
        Ew = [sbt(f"Ew{i}", [P, C * BWMAX], bf16) for i in range(NBUF)]
        rbl = [sbt(f"rb{i}", [P, BWMAX], bf16) for i in range(NBUF)]
        zw = sbt("zw", [P, 8 * BWMAX], bf16)
        zf = sbt("zf", [P, BWMAX], fp32)
        rf = sbt("rf", [P, BWMAX], fp32)
        onesw = sbt("onesw", [P, 1], bf16)
        ltov = sbt("ltov_sb", [P, OV], bf16)
        etov = sbt("etov", [P, OV], bf16)
        gov = sbt("gov", [P, OV], bf16)
        osb = sbt("osb", [P, 704], fp32)
        acc = pst("acc", [G, C * G], fp32)
        acc2 = pst("acc2", [P, IW], fp32)
        acch = pst("acch", [1, OV], fp32)

        # exp table + bias-const preload FIRST: creating the activation
        # bias const mid-stream emits an all-engine const-setup barrier,
        # which must precede the (waiting) DMA queue or it deadlocks
        nc.scalar.activation(zw[:, 0:1], zw[:, 0:1], X)

        # ---- Sync: all DMAs ----
        nc.sync.dma_start(
            Lb[0][:, 0 : C * BLOCKS[0]], logits_d[0].ap()
        ).then_inc(sd[0], 16)
        nc.sync.dma_start(ltov[:], ltov_d.ap()).then_inc(sdL, 16)
        for b in range(1, NBLK):
            d = nc.sync.dma_start(
                Lb[b % NLBUF][:, 0 : C * BLOCKS[b]], logits_d[b].ap()
            )
            if b >= NLBUF:  # Lb[b%NLBUF] is free once exp(b-NLBUF) read it
                d._wait_ge(s_act, b - NLBUF + 1)
            d.then_inc(sd[b], 16)
        dout = nc.sync.dma_start(out_d.ap(), osb[:])
        dout._wait_ge(s_drain, 1)
        dout.then_inc(s_nul, 16)

        # ---- Scalar: one exp per block + drains ----
        for b in range(NBLK):
            if b >= NBUF:  # Ew[b%NBUF] free once block b-NBUF matmuls done
                nc.scalar.sem_inc(s_nul, 1)._wait_ge(s_pe, b - NBUF + 1)
            nc.scalar.activation(
                Ew[b % NBUF][:, 0 : C * BLOCKS[b]],
                Lb[b % NLBUF][:, 0 : C * BLOCKS[b]],
                X,
            )._wait_ge(sd[b], 16).then_inc(s_act, 1)
            if b == 0:
                nc.scalar.activation(etov[:], ltov[:], X)._wait_ge(sdL, 16)
        # PSUM drains (Scalar idle after its last exp; ScE is near PSUM)
        nc.scalar.copy(osb[0:1, C * G + IW :], acch[0:1, :])._wait_ge(s_pe, NBLK)
        nc.scalar.copy(osb[0:G, 0 : C * G], acc[:])
        nc.scalar.copy(osb[:, C * G : C * G + IW], acc2[:]).then_inc(s_drain, 1)

        # ---- Vector: memsets + per-block Z tree, recip, cast (+ gov) ----
        tt = nc.vector.tensor_tensor
        nc.vector.memset(onesw[:], 1.0)
        nc.vector.memset(osb[:], 0.0)
        for b in range(NBLK):
            i = b % NBUF
            bw = BLOCKS[b]
            flat = Ew[i][:, 0 : C * bw]
            E4 = flat.rearrange("p (a c j) -> p a c j", c=C, j=16)
            z4 = zw[:, 0 : 8 * bw].rearrange("p (a s j) -> p a s j", s=8, j=16)
            if b >= NBUF:  # rbl[i] free once block b-NBUF matmuls done
                nc.vector.sem_inc(s_nul, 1)._wait_ge(s_pe, b - NBUF + 1)
            tt(z4[:, :, :, :], E4[:, :, 0:8, :], E4[:, :, 8:16, :], AL.add
               )._wait_ge(s_act, b + 1)
            tt(z4[:, :, 0:4, :], z4[:, :, 0:4, :], z4[:, :, 4:8, :], AL.add)
            tt(z4[:, :, 0:2, :], z4[:, :, 0:2, :], z4[:, :, 2:4, :], AL.add)
            tt(zf[:, 0:bw], z4[:, :, 0, :], z4[:, :, 1, :], AL.add)
            nc.vector.reciprocal_approx_fast(rf[:, 0:bw], zf[:, 0:bw])
            nc.vector.tensor_copy(rbl[i][:, 0:bw], rf[:, 0:bw]).then_inc(s_dve, 1)
            if b == 0:
                # overflow: g = exp(lt_ov) * r (etov ready once s_act>=2,
                # which block 1's L1 waits on anyway)
                tt(gov[:], etov[:], rbl[0][:, 0:OV], AL.mult)._wait_ge(
                    s_act, 2
                ).then_inc(s_gov, 1)

        # ---- Tensor: per-block PS chunks + I pieces (+ overflow h-sum) ----
        for b in range(NBLK):
            i = b % NBUF
            bw = BLOCKS[b]
            flat = Ew[i][:, 0 : C * bw]
            Epr = flat.rearrange("p (a r) -> p a r", r=16 * C)
            nc.tensor.sem_inc(s_nul, 1)._wait_ge(s_dve, b + 1)
            for k in range(bw // G):
                m = BOFF[b] // G + k
                jc = k * G
                nc.tensor.matmul(
                    acc[:, :],
                    rbl[i][:, jc : jc + G],
                    flat[:, jc * C : (jc + G) * C],
                    start=m == 0,
                    stop=m == NCH - 1,
                    skip_group_check=True,
                )
            if b == 0:
                nc.tensor.matmul(
                    acch[:, :], onesw[:, 0:1], gov[:, :],
                    start=True, stop=True, skip_group_check=True,
                )._wait_ge(s_gov, 1).then_inc(s_pe, 1)
            else:
                lo = 0
                last = None
                while lo < bw:
                    w = min(IW, bw - lo)
                    last = nc.tensor.matmul(
                        acc2[0:w, 0:w],
                        rbl[i][:, lo : lo + w],
                        Epr[:, lo // 16 : (lo + w) // 16, 0 : 16 * C : C + 1],
                        start=BOFF[b] + lo == OV,
                        stop=BOFF[b] + lo + w == M,
                        skip_group_check=True,
                    )
                    lo += w
                last.then_inc(s_pe, 1)
        nc.all_engine_barrier()
    nc.compile()
    return nc


_NC_CACHE = {}


def _get_nc():
    if "nc" not in _NC_CACHE:
        _NC_CACHE["nc"] = build()
    return _NC_CACHE["nc"]


def _prep_core(lg, t):
    """lg [C, N] fp32, t [N] int -> device inputs + host metadata."""
    cnts = np.bincount(t, minlength=C)
    order = np.argsort(t, kind="stable")
    offs = np.concatenate([[0], np.cumsum(cnts)])

    vox = np.full(M * P, -1, dtype=np.int64)   # (column j, slot p) major
    qdum = np.zeros(C, dtype=np.int64)         # dummies inside class quotas
    ov_cm = np.zeros(OV, dtype=np.int64)       # overflow column class map
    ovdum = np.zeros(C, dtype=np.int64)        # dummies in overflow columns
    ov_col = 0
    for c in range(C):
        n_c = int(cnts[c])
        n_q = min(n_c, Q * P)
        idx = np.arange(n_q)
        # class c quota voxel a*P+p -> global column OV + 16*a + c, slot p
        vox[(OV + C * (idx // P) + c) * P + idx % P] = order[
            offs[c] : offs[c] + n_q
        ]
        qdum[c] = Q * P - n_q
        n_x = n_c - n_q
        if n_x > 0:
            ncols = (n_x + P - 1) // P
            base = ov_col * P
            vox[base : base + n_x] = order[offs[c] + n_q : offs[c] + n_c]
            ov_cm[ov_col : ov_col + ncols] = c
            ovdum[c] += ncols * P - n_x
            ov_col += ncols
    assert ov_col <= OV, f"overflow region too small: {ov_col} > {OV}"
    ovdum[0] += (OV - ov_col) * P  # trailing all-dummy columns, class 0

    mask = vox >= 0
    A = lg[:, np.clip(vox, 0, None)]  # [C, M*P]
    A[:, ~mask] = 0.0
    A3 = A.reshape(C, M, P)
    im = {}
    for b in range(NBLK):
        bw = BLOCKS[b]
        # hybrid layout [p][a][c][j1]: a = 16-col group, j1 = col % 16
        seg = A3[:, BOFF[b] : BOFF[b + 1], :].reshape(C, bw // 16, 16, P)
        im[f"l{b}"] = np.ascontiguousarray(
            seg.transpose(3, 1, 0, 2)
        ).reshape(P, C * bw).astype(ml_dtypes.float8_e4m3)
    ovj = np.arange(OV)
    ltov = A3[ov_cm, ovj, :]  # [OV, P] target-class logits of ov columns
    im["ltov"] = np.ascontiguousarray(ltov.T).astype(ml_dtypes.bfloat16)
    return im, (qdum, ov_cm, ovdum, cnts)


def shard_inputs(logits, targets):
    """Core i gets batch i//4, d-slab i%4. Returns (in_maps, metas)."""
    in_maps, metas = [], []
    for i in range(NCORES):
        b, q = divmod(i, 4)
        lg = np.ascontiguousarray(
            logits[b, :, q * DSH : (q + 1) * DSH], dtype=np.float32
        ).reshape(C, N)
        t = np.ascontiguousarray(
            targets[b, q * DSH : (q + 1) * DSH], dtype=np.int64
        ).reshape(N)
        im, meta = _prep_core(lg, t)
        in_maps.append(im)
        metas.append(meta)
    return in_maps, metas


def _core_stats(res, meta):
    """Per-core (I, PS, counts) from device outputs + host metadata."""
    qdum, ov_cm, ovdum, cnts = meta
    o = res["o"].astype(np.float64)
    gidx = np.arange(G)
    fbase = (gidx // 16) * 256 + gidx % 16  # chunk-slab offset of column g
    PS = o[gidx[:, None], fbase[:, None] + 16 * np.arange(C)[None, :]].sum(axis=0)
    PS -= (qdum.sum() + ovdum.sum()) / 16.0  # dummies add e*r = 1/16 everywhere
    d2 = o[np.arange(P), C * G + np.arange(IW)]  # I diag, class g%16
    I = d2.reshape(-1, C).sum(axis=0) - qdum / 16.0
    hov = o[0, C * G + IW :]
    I += np.bincount(ov_cm, weights=hov, minlength=C)[:C] - ovdum / 16.0
    return I, PS, cnts.astype(np.float64)


def kernel(logits, targets):
    logits = np.asarray(logits)
    targets = np.asarray(targets)
    nc = _get_nc()
    in_maps, metas = shard_inputs(logits, targets)
    res = run_bass_kernel_spmd(nc, in_maps, list(range(NCORES))).results
    inter = np.zeros((B, C))
    probs_sum = np.zeros((B, C))
    counts = np.zeros((B, C))
    for i in range(NCORES):
        I, PS, CNT = _core_stats(res[i], metas[i])
        inter[i // 4] += I
        probs_sum[i // 4] += PS
        counts[i // 4] += CNT
    dice = (2.0 * inter + SMOOTH) / (probs_sum + counts + SMOOTH)
    mask = np.ones(C)
    mask[IGNORE_INDEX] = 0.0
    mean_dice = (dice * mask[None, :]).sum() / (B * (C - 1))
    return np.float32(1.0 - mean_dice)
